# revision 1
# baseline (speedup 1.0000x reference)
"""GAT (2-layer, PyG-style) Trainium2 Bass kernel, 8 NeuronCores.

Strategy (dst-sharded, slot-major, gather-based):
- Nodes ranked by in-degree, tiled into 392 global tiles of 128 lanes;
  core(g)=g%8, tile(g)=g//8 -> each core owns 49 dst tiles (6272 slots,
  50176 total rows incl. 176 fakes). newid = core*6272 + tile*128 + lane.
- conv tables in HBM, 256B-pitch rows (bf16):
    table1 row: [feats1 c-major(64) | alpha_s1(8)] (+pad)
    table2 row: [feats2 perm(40) | alpha_s2(1)] (+pad)
- Edges processed slot-major: round r of tile t gathers the r-th in-edge
  src row for each of the 128 dst lanes (dma_gather, int16 idx).
  int16 range forces an A/B table split at row 32768: pass A covers
  in-edges with src row < 32768 (per-tile K_A rounds, padded to the max
  lane count over all cores), pass B the rest. Pads point at PAD rows
  whose alpha_s = -3e4 => gate exp(leakyrelu(...)) == 0 exactly.
- Aggregation: per round, DVE scales gathered feats by g = exp(lrelu(
  alpha_s[src]+alpha_d[dst])) (c-major broadcast keeps DVE 2x mode), and
  an identity-weight matmul accumulates [g*f | g] into the tile's PSUM:
  numerator and softmax denominator in one pass. Pass A results are
  parked in SBUF f32 and added back during pass B's epilogue.
- Between convs: each core builds its table2 shard (transpose + matmul
  with W2ext), patches its fakes' alpha_s2, AllGathers the tight shard,
  and repacks to 256B pitch.
"""

import numpy as np
import ml_dtypes

import concourse.bass as bass
import concourse.bacc as bacc
import concourse.mybir as mybir
from concourse.tile import TileContext
from concourse.masks import make_identity
from concourse.bass_utils import run_bass_kernel_spmd

bf16 = ml_dtypes.bfloat16
FP = mybir.dt.float32
BF = mybir.dt.bfloat16
I16 = mybir.dt.int16

N = 50000
E = 1_600_000
F_IN = 128
H, C1 = 8, 8
D1 = 64
NC_ = 40                 # num classes
NEG = 0.2
NCORES = 8
NTILES = 49
NSH = NTILES * 128       # 6272
NTOT = NCORES * NSH      # 50176
SPLIT = 32768            # table A/B boundary (int16 idx range)
PITCH = 128              # table row pitch in bf16 elements (256B)
ROW1 = 72                # gathered row width conv1 (feats 64 + alpha_s 8)
ROW2 = 41                # conv2 (feats 40 + alpha_s 1)
ANEG = -30000.0
MAXG = 8192              # max idxs per dma_gather (64 rounds)


# --------------------------------------------------------------------------
# host planning
# --------------------------------------------------------------------------

def _plan(edge_index):
    src = np.asarray(edge_index[0], np.int64)
    dst = np.asarray(edge_index[1], np.int64)
    loops = np.arange(N, dtype=np.int64)
    src = np.concatenate([src, loops])
    dst = np.concatenate([dst, loops])

    indeg = np.bincount(dst, minlength=N)
    order = np.argsort(-indeg, kind="stable")          # rank -> node
    ranks = np.arange(NTOT)
    g = ranks // 128
    newid_of_rank = (g % NCORES) * NSH + (g // NCORES) * 128 + ranks % 128
    newid = np.empty(N, np.int64)
    newid[order] = newid_of_rank[:N]

    # make row 6250 a fake (A-half PAD row): move its real node to a B fake
    r0 = int(np.where(newid == 6250)[0][0]) if (newid == 6250).any() else -1
    if r0 >= 0:
        newid[r0] = 50048
    e_src_row = newid[src]
    e_dst_new = newid[dst]
    e_core = e_dst_new // NSH
    e_rem = e_dst_new % NSH
    e_t = e_rem // 128
    e_lane = e_rem % 128
    e_isA = e_src_row < SPLIT

    # per (core, tile, lane) counts of A / B in-edges
    flat_lane = (e_core * NTILES + e_t) * 128 + e_lane
    cntA = np.bincount(flat_lane[e_isA], minlength=NCORES * NTILES * 128)
    cntB = np.bincount(flat_lane[~e_isA], minlength=NCORES * NTILES * 128)
    cntA = cntA.reshape(NCORES, NTILES, 128)
    cntB = cntB.reshape(NCORES, NTILES, 128)
    KA = cntA.max(axis=(0, 2)).astype(np.int64)        # per-tile common
    KB = cntB.max(axis=(0, 2)).astype(np.int64)
    KA = np.maximum(KA, 1)
    KB = np.maximum(KB, 1)
    baseA = np.concatenate([[0], np.cumsum(KA)])
    baseB = np.concatenate([[0], np.cumsum(KB)])
    RA, RB = int(baseA[-1]), int(baseB[-1])

    # slot assignment: order edges by (phase-stream position)
    PAD_A = 6250                                       # core0 fake (A half)
    PAD_B = 7 * NSH + 6250                             # core7 fake (B half)
    slotA = np.full((NCORES, 128, RA), PAD_A, np.int32)
    slotB = np.full((NCORES, 128, RB), PAD_B - SPLIT, np.int32)

    # cumcount within (core,tile,lane,phase)
    key = flat_lane * 2 + (~e_isA)
    sidx = np.argsort(key, kind="stable")
    ks = key[sidx]
    newgrp = np.ones(len(ks), bool)
    newgrp[1:] = ks[1:] != ks[:-1]
    pos = np.arange(len(ks))
    start = np.maximum.accumulate(np.where(newgrp, pos, 0))
    cum = pos - start
    slot = np.empty(len(ks), np.int64)
    slot[sidx] = cum

    mA = e_isA
    slotA[e_core[mA], e_lane[mA], baseA[e_t[mA]] + slot[mA]] = e_src_row[mA]
    mB = ~e_isA
    slotB[e_core[mB], e_lane[mB], baseB[e_t[mB]] + slot[mB]] = (
        e_src_row[mB] - SPLIT)

    # chunks: split pass streams at MAXG//128-round boundaries
    CR = MAXG // 128
    def mk_chunks(K, base, Rtot):
        chunks = []   # (r0, nr, segments=[(tile, seg_r0_global, seg_nr, tile_r0, tile_done)])
        r = 0
        while r < Rtot:
            nr = min(CR, Rtot - r)
            segs = []
            for t in range(NTILES):
                s0, s1 = int(base[t]), int(base[t + 1])
                a, b = max(s0, r), min(s1, r + nr)
                if a < b:
                    segs.append((t, a, b - a, a - s0, b == s1))
            chunks.append((r, nr, segs))
            r += nr
        return chunks
    chunksA = mk_chunks(KA, baseA, RA)
    chunksB = mk_chunks(KB, baseB, RB)

    # idx stream int16 [NCORES, 128, NW]: per chunk block of nr*8 cols
    def mk_idx(slots, chunks):
        blocks = []
        for (r0, nr, _) in chunks:
            # list position i = (r-r0)*128 + lane ; value slots[:, lane, r]
            blk = slots[:, :, r0:r0 + nr]              # [8, 128, nr]
            flat = blk.transpose(0, 2, 1).reshape(NCORES, nr * 128)
            cols = nr * 8
            w = np.zeros((NCORES, 16, cols), np.int16)
            ii = np.arange(nr * 128)
            w[:, ii % 16, ii // 16] = flat
            blocks.append(np.tile(w, (1, 8, 1)))       # replicate to 128 parts
        return np.concatenate(blocks, axis=2)          # [8, 128, NW]
    idxA = mk_idx(slotA, chunksA)
    idxB = mk_idx(slotB, chunksB)
    idx_all = np.concatenate([idxA, idxB], axis=2)
    NWA = idxA.shape[2]

    return dict(order=order, newid=newid, KA=KA, KB=KB, chunksA=chunksA,
                chunksB=chunksB, idx=idx_all, NWA=NWA, RA=RA, RB=RB)


# --------------------------------------------------------------------------
# gather instruction (tight rows on a 256B pitch; bypasses bass' %256 check)
# --------------------------------------------------------------------------

def _gather(eng, out_ap, in_ap, idxs_ap, num_idxs, elem_size, elem_step,
            queue_num=0):
    dts = mybir.dt.size(in_ap.dtype)
    sb = elem_step * dts
    assert sb % 256 == 0 and sb // 256 < 256
    _in = eng.lower_ap_dma(in_ap, for_custom_bir_dma=True)
    return eng.add_instruction(
        mybir.InstDMAGatherAnt(
            name=eng.bass.get_next_instruction_name(),
            ins=[*_in, eng.lower_ap(idxs_ap),
                 eng.lower_val_access(eng.to_reg(num_idxs))],
            outs=[eng.lower_ap(out_ap)],
            transpose=False, num_idxs=num_idxs, elem_size=elem_size,
            stride_bytes_256=sb // 256, gen_mode=0, single_packet=False,
            queue_num=queue_num, sbuf_tokens_per_rank=0, sbuf_free_dim_per_rank=0,
            sbuf_free_dim_pad_per_rank=0, sbuf_byte_offset=0,
        ))


def _bc(ap, dims):
    """Hand-built broadcast AP: dims = list of [step, count]."""
    return bass.AP(ap.tensor, ap.offset, dims)


def _dram3(handle, j0, nchunk, width, pitch):
    """DRAM AP [p=128, a=nchunk, e=width] with row = j0 + a*128 + p."""
    ap = handle[:]
    return bass.AP(ap.tensor, j0 * pitch,
                   [[pitch, 128], [128 * pitch, nchunk], [1, width]])


# --------------------------------------------------------------------------
# device program
# --------------------------------------------------------------------------

def _build(plan):
    KA, KB = plan["KA"], plan["KB"]
    chunksA, chunksB = plan["chunksA"], plan["chunksB"]
    NW = plan["idx"].shape[2]
    NWA = plan["NWA"]

    nc = bacc.Bacc("TRN2", num_devices=NCORES, num_swdge_queues=2)
    AF = mybir.ActivationFunctionType

    xT = nc.dram_tensor("xT", [F_IN, NTOT], BF, kind="ExternalInput")
    xTo = nc.dram_tensor("xTo", [F_IN, NSH], BF, kind="ExternalInput")
    W1e = nc.dram_tensor("W1e", [F_IN, 80], BF, kind="ExternalInput")
    W2e = nc.dram_tensor("W2e", [D1, 42], BF, kind="ExternalInput")
    b1r = nc.dram_tensor("b1r", [128, D1], FP, kind="ExternalInput")
    b2r = nc.dram_tensor("b2r", [128, NC_], FP, kind="ExternalInput")
    idx = nc.dram_tensor("idx", [128, NW], I16, kind="ExternalInput")
    out = nc.dram_tensor("out", [NTILES, 128, NC_], FP, kind="ExternalOutput")

    tab1 = nc.dram_tensor("tab1", [NTOT, PITCH], BF, kind="Internal")
    shard2 = nc.dram_tensor("shard2", [NSH, 42], BF, kind="Internal")
    tab2t = nc.dram_tensor("tab2t", [NTOT, 42], BF, kind="Internal",
                           addr_space="Shared")
    tab2 = nc.dram_tensor("tab2", [NTOT, PITCH], BF, kind="Internal")

    with TileContext(nc, num_cores=NCORES) as tc:
        with (
            tc.tile_pool(name="const", bufs=1) as const,
            tc.tile_pool(name="io", bufs=3) as io,
            tc.tile_pool(name="work", bufs=4) as work,
            tc.tile_pool(name="ps_acc", bufs=2, space="PSUM") as ps_acc,
            tc.tile_pool(name="ps_b", bufs=2, space="PSUM") as ps_b,
            tc.tile_pool(name="ps_e", bufs=1, space="PSUM") as ps_e,
        ):
            idf = const.tile([128, 128], FP, name="idf")
            make_identity(nc, idf[:])
            idb = const.tile([128, 128], BF, name="idb")
            nc.vector.tensor_copy(out=idb[:], in_=idf[:])
            w1 = const.tile([F_IN, 80], BF, name="w1")
            nc.sync.dma_start(out=w1[:], in_=W1e[:])
            w2 = const.tile([D1, 42], BF, name="w2")
            nc.sync.dma_start(out=w2[:], in_=W2e[:])
            b1t = const.tile([128, D1], FP, name="b1t")
            nc.sync.dma_start(out=b1t[:], in_=b1r[:])
            b2t = const.tile([128, NC_], FP, name="b2t")
            nc.sync.dma_start(out=b2t[:], in_=b2r[:])
            negt = const.tile([128, 1], BF, name="negt")
            nc.gpsimd.memset(negt[:], ANEG)
            idx_t = const.tile([128, NW], I16, name="idx_t")
            nc.sync.dma_start(out=idx_t[:], in_=idx[:])
            ad1 = const.tile([128, NTILES * 8], FP, name="ad1")
            ad2 = const.tile([128, NTILES], FP, name="ad2")
            accA1 = const.tile([128, NTILES * ROW1], FP, name="accA1")
            accA2 = const.tile([128, NTILES * ROW2], FP, name="accA2")

            # ---- phase 1: replicated table1 build -------------------------
            XB = 512
            for j0 in range(0, NTOT, XB):
                xt = io.tile([128, XB], BF, tag="xt", name="xt")
                nc.sync.dma_start(out=xt[:], in_=xT[:, j0:j0 + XB])
                pb = ps_b.tile([128, 4 * ROW1], FP, tag="pb", name="pb")
                st = io.tile([128, 4 * ROW1], BF, tag="st", name="st")
                for k in range(4):
                    nc.tensor.matmul(
                        out=pb[:, k * ROW1:(k + 1) * ROW1],
                        lhsT=xt[:, k * 128:(k + 1) * 128],
                        rhs=w1[:, :ROW1], start=True, stop=True)
                eng = nc.vector if (j0 // XB) % 2 == 0 else nc.scalar
                if eng is nc.vector:
                    eng.tensor_copy(out=st[:], in_=pb[:])
                else:
                    eng.activation(st[:], pb[:], AF.Copy)
                nc.sync.dma_start(
                    out=_dram3(tab1, j0, 4, ROW1, PITCH), in_=st[:])

            # ---- phase 1b: own alpha_d1 ----------------------------------
            for t in range(NTILES):
                xo = io.tile([128, 128], BF, tag="xo", name="xo")
                nc.sync.dma_start(out=xo[:], in_=xTo[:, t * 128:(t + 1) * 128])
                pa = ps_e.tile([128, 8], FP, tag="pa", name="pa")
                nc.tensor.matmul(out=pa[:], lhsT=xo[:], rhs=w1[:, 72:80],
                                 start=True, stop=True)
                nc.vector.tensor_copy(out=ad1[:, t * 8:(t + 1) * 8], in_=pa[:])

            # ---- conv passes ---------------------------------------------
            def conv_pass(conv, phase, chunks, col0, tab, split_base):
                ROW = ROW1 if conv == 1 else ROW2
                psums = {}
                for ci, (r0, nr, segs) in enumerate(chunks):
                    nidx = nr * 128
                    cw = nr * 8
                    buf = work.tile([128, nr, ROW], BF, tag=f"g{conv}", name=f"buf{conv}")
                    src_ap = tab[split_base:split_base + SPLIT, :ROW] \
                        if split_base == 0 else tab[SPLIT:, :ROW]
                    _gather(nc.gpsimd, buf[:], src_ap,
                            idx_t[:, col0 + r0 * 8: col0 + r0 * 8 + cw],
                            nidx, ROW, PITCH, queue_num=ci % 2)
                    # e = alpha_s + alpha_d per segment; prelu+exp chunk-wide
                    if conv == 1:
                        e = work.tile([128, nr, 8], FP, tag="e1", name="e1")
                        gg = work.tile([128, nr, 8], BF, tag="gg1", name="gg1")
                        for (t, a, n, tr0, _) in segs:
                            o = a - r0
                            adv = ad1[:, t * 8:t * 8 + 8]
                            nc.vector.tensor_tensor(
                                out=e[:, o:o + n, :],
                                in0=buf[:, o:o + n, 64:72],
                                in1=_bc(adv[:], [adv[:].ap[0], [0, n], [1, 8]]),
                                op=mybir.AluOpType.add)
                        es = work.tile([128, nr, 8], FP, tag="es1", name="es1")
                        nc.vector.tensor_scalar(es[:], e[:], NEG, None,
                                                mybir.AluOpType.mult)
                        nc.vector.tensor_tensor(out=e[:], in0=e[:], in1=es[:],
                                                op=mybir.AluOpType.max)
                        nc.scalar.activation(gg[:], e[:], AF.Exp)
                        gb = gg[:]
                        bb = buf[:]
                        b4 = _bc(bb, [bb.ap[0], [ROW, nr], [8, 8], [1, 8]])
                        nc.vector.tensor_tensor(
                            out=b4, in0=b4,
                            in1=_bc(gb, [gb.ap[0], [8, nr], [0, 8], [1, 8]]),
                            op=mybir.AluOpType.mult)
                        nc.vector.tensor_copy(out=buf[:, :, 64:72], in_=gg[:])
                    else:
                        e = work.tile([128, nr, 1], FP, tag="e2", name="e2")
                        gg = work.tile([128, nr, 1], BF, tag="gg2", name="gg2")
                        g8 = work.tile([128, nr, 8], BF, tag="g8", name="g8")
                        for (t, a, n, tr0, _) in segs:
                            o = a - r0
                            adv = ad2[:, t:t + 1]
                            nc.vector.tensor_tensor(
                                out=e[:, o:o + n, :],
                                in0=buf[:, o:o + n, 40:41],
                                in1=_bc(adv[:], [adv[:].ap[0], [0, n], [0, 1]]),
                                op=mybir.AluOpType.add)
                        es = work.tile([128, nr, 1], FP, tag="es2", name="es2")
                        nc.vector.tensor_scalar(es[:], e[:], NEG, None,
                                                mybir.AluOpType.mult)
                        nc.vector.tensor_tensor(out=e[:], in0=e[:], in1=es[:],
                                                op=mybir.AluOpType.max)
                        nc.scalar.activation(gg[:], e[:], AF.Exp)
                        gb = gg[:]
                        nc.vector.tensor_copy(
                            out=g8[:],
                            in_=_bc(gb, [gb.ap[0], [1, nr], [0, 8]]))
                        g8b = g8[:]
                        bb = buf[:]
                        b4 = _bc(bb, [bb.ap[0], [ROW, nr], [8, 5], [1, 8]])
                        nc.vector.tensor_tensor(
                            out=b4, in0=b4,
                            in1=_bc(g8b, [g8b.ap[0], [8, nr], [0, 5], [1, 8]]),
                            op=mybir.AluOpType.mult)
                        nc.vector.tensor_copy(out=buf[:, :, 40:41], in_=gg[:])
                    # accumulate rounds into per-tile psum
                    for (t, a, n, tr0, done) in segs:
                        if t not in psums:
                            psums[t] = ps_acc.tile([128, ROW1], FP, tag="acc", name=f"ps{conv}_{t}")
                        pt = psums[t]
                        for r in range(n):
                            nc.tensor.matmul(
                                out=pt[:, :ROW], lhsT=idb[:, :128],
                                rhs=buf[:, a - r0 + r, :],
                                start=(tr0 + r == 0), stop=done and r == n - 1)
                        if done:
                            yield t, pt
                            del psums[t]

            accv1 = accA1[:].rearrange("p (t e) -> p t e", t=NTILES)
            for t, pt in conv_pass(1, "A", chunksA, 0, tab1, 0):
                nc.vector.tensor_copy(out=accv1[:, t, :], in_=pt[:, :ROW1])

            h1s = {}
            for t, pt in conv_pass(1, "B", chunksB, NWA, tab1, SPLIT):
                nd = work.tile([128, ROW1], FP, tag="nd1", name="nd1")
                nc.vector.tensor_tensor(out=nd[:], in0=accv1[:, t, :],
                                        in1=pt[:, :ROW1], op=mybir.AluOpType.add)
                den = work.tile([128, 8], FP, tag="den1", name="den1")
                nc.vector.tensor_scalar(den[:], nd[:, 64:72], 1e-16, None,
                                        mybir.AluOpType.max)
                rec = work.tile([128, 8], FP, tag="rec1", name="rec1")
                nc.vector.reciprocal(rec[:], den[:])
                h1 = work.tile([128, D1], FP, tag="h1", name="h1")
                rb = rec[:]
                h1v = h1[:]
                ndv = nd[:]
                nc.vector.tensor_tensor(
                    out=_bc(h1v, [h1v.ap[0], [8, 8], [1, 8]]),
                    in0=_bc(ndv, [ndv.ap[0], [8, 8], [1, 8]]),
                    in1=_bc(rb, [rb.ap[0], [0, 8], [1, 8]]),
                    op=mybir.AluOpType.mult)
                nc.vector.tensor_tensor(out=h1[:], in0=h1[:], in1=b1t[:],
                                        op=mybir.AluOpType.add)
                nc.vector.tensor_scalar(h1[:], h1[:], 0.0, None,
                                        mybir.AluOpType.max)
                ptr = ps_e.tile([64, 128], FP, tag="tr", name="ptr")
                nc.tensor.transpose(out=ptr[:], in_=h1[:], identity=idf[:])
                h1T = work.tile([64, 128], BF, tag="h1T", name="h1T")
                nc.vector.tensor_copy(out=h1T[:], in_=ptr[:])
                pf2 = ps_e.tile([128, 42], FP, tag="pf2", name="pf2")
                nc.tensor.matmul(out=pf2[:], lhsT=h1T[:], rhs=w2[:],
                                 start=True, stop=True)
                nc.vector.tensor_copy(out=ad2[:, t:t + 1], in_=pf2[:, 41:42])
                st2 = work.tile([128, 42], BF, tag="st2", name="st2")
                nc.vector.tensor_copy(out=st2[:], in_=pf2[:])
                nc.sync.dma_start(out=shard2[t * 128:(t + 1) * 128, :],
                                  in_=st2[:])

            # allgather, repack to 256B pitch
            nc.gpsimd.collective_compute(
                "AllGather", mybir.AluOpType.bypass,
                replica_groups=[list(range(NCORES))],
                ins=[shard2[:]], outs=[tab2t[:]])
            RPB = 1024
            for j0 in range(0, NTOT, RPB):
                rp = io.tile([128, 8 * ROW2], BF, tag="rp", name="rp")
                nc.sync.dma_start(out=rp[:], in_=_dram3(tab2t, j0, 8, ROW2, 42))
                nc.sync.dma_start(out=_dram3(tab2, j0, 8, ROW2, PITCH), in_=rp[:])
            # patch all fake rows' alpha_s2 (global newids, same on all cores)
            nc.sync.dma_start(out=tab2[6250:6251, 40:41], in_=negt[:1])
            nc.sync.dma_start(out=tab2[43856:43904, 40:41], in_=negt[:48])
            nc.sync.dma_start(out=tab2[50049:50176, 40:41], in_=negt[:127])

            accv2 = accA2[:].rearrange("p (t e) -> p t e", t=NTILES)
            for t, pt in conv_pass(2, "A", chunksA, 0, tab2, 0):
                nc.vector.tensor_copy(out=accv2[:, t, :], in_=pt[:, :ROW2])

            for t, pt in conv_pass(2, "B", chunksB, NWA, tab2, SPLIT):
                nd = work.tile([128, ROW2], FP, tag="nd2", name="nd2")
                nc.vector.tensor_tensor(out=nd[:], in0=accv2[:, t, :],
                                        in1=pt[:, :ROW2], op=mybir.AluOpType.add)
                den = work.tile([128, 1], FP, tag="den2", name="den2")
                nc.vector.tensor_scalar(den[:], nd[:, 40:41], 1e-16, None,
                                        mybir.AluOpType.max)
                rec = work.tile([128, 1], FP, tag="rec2", name="rec2")
                nc.vector.reciprocal(rec[:], den[:])
                o2 = work.tile([128, NC_], FP, tag="o2", name="o2")
                nc.vector.tensor_scalar(o2[:], nd[:, 0:40], rec[:, 0:1], None,
                                        mybir.AluOpType.mult)
                nc.vector.tensor_tensor(out=o2[:], in0=o2[:], in1=b2t[:],
                                        op=mybir.AluOpType.add)
                mx = work.tile([128, 1], FP, tag="mx", name="mx")
                nc.vector.tensor_reduce(mx[:], o2[:], mybir.AxisListType.X,
                                        mybir.AluOpType.max)
                nc.vector.tensor_scalar(o2[:], o2[:], mx[:, 0:1], None,
                                        mybir.AluOpType.subtract)
                ex = work.tile([128, NC_], FP, tag="ex", name="ex")
                sm = work.tile([128, 1], FP, tag="sm", name="sm")
                nc.scalar.activation(ex[:], o2[:], AF.Exp, accum_out=sm[:])
                ls = work.tile([128, 1], FP, tag="ls", name="ls")
                nc.scalar.activation(ls[:], sm[:], AF.Ln)
                nc.vector.tensor_scalar(o2[:], o2[:], ls[:, 0:1], None,
                                        mybir.AluOpType.subtract)
                nc.sync.dma_start(out=out[t], in_=o2[:])

    nc.finalize()
    return nc


# --------------------------------------------------------------------------
# host entry
# --------------------------------------------------------------------------

def kernel(x, edge_index, W1, as1, ad1, b1, W2, as2, ad2, b2):
    x = np.asarray(x, np.float32)
    ei = np.asarray(edge_index)
    W1 = np.asarray(W1, np.float32); as1 = np.asarray(as1, np.float32)
    ad1 = np.asarray(ad1, np.float32); b1 = np.asarray(b1, np.float32)
    W2 = np.asarray(W2, np.float32); as2 = np.asarray(as2, np.float32)
    ad2 = np.asarray(ad2, np.float32); b2 = np.asarray(b2, np.float32)

    plan = _plan(ei)
    newid, order = plan["newid"], plan["order"]

    # W1ext: [128, 80] = [W1 c-major | W1@as1_h | W1@ad1_h]
    W1cm = W1.reshape(F_IN, H, C1).transpose(0, 2, 1).reshape(F_IN, D1)
    Was = np.stack([W1[:, h * C1:(h + 1) * C1] @ as1[h] for h in range(H)], 1)
    Wad = np.stack([W1[:, h * C1:(h + 1) * C1] @ ad1[h] for h in range(H)], 1)
    W1e = np.concatenate([W1cm, Was, Wad], axis=1).astype(bf16)

    # x_fake: alpha_s1 = ANEG (all heads), alpha_d1 = 0
    M = np.concatenate([Was, Wad], axis=1)                    # [128, 16]
    rhs = np.concatenate([np.full(8, ANEG), np.zeros(8)])
    v = M @ np.linalg.solve(M.T @ M, rhs)
    xT_all = np.zeros((F_IN, NTOT), np.float32)
    xT_all[:, newid] = x.T
    fake = np.setdiff1d(np.arange(NTOT), newid)
    xT_all[:, fake] = v[:, None]
    xT_all = xT_all.astype(bf16)

    # conv2: fake-head col permutation: new col j=c*8+h <-> orig 8c? no:
    # orig col o in [0,40): treat as (h,c5): o = h*5+c ; new j = c*8+h
    sig = np.empty(NC_, np.int64)
    for hh in range(8):
        for cc in range(5):
            sig[cc * 8 + hh] = hh * 5 + cc
    W2p = W2[:, sig]
    W2ex = np.concatenate([W2p, W2 @ as2[0][:, None], W2 @ ad2[0][:, None]],
                          axis=1)                             # [64, 42]
    # h1 columns are c-major (c*8+h); permute W2ext rows to match
    rowperm = np.empty(D1, np.int64)
    for hh in range(H):
        for cc in range(C1):
            rowperm[cc * 8 + hh] = hh * C1 + cc
    W2ex = W2ex[rowperm].astype(bf16)

    b1cm = b1.reshape(H, C1).T.reshape(D1)
    b1r = np.tile(b1cm, (128, 1)).astype(np.float32)
    b2r = np.tile(b2[sig], (128, 1)).astype(np.float32)

    nc = _build(plan)
    in_maps = []
    for c in range(NCORES):
        in_maps.append({
            "xT": xT_all,
            "xTo": np.ascontiguousarray(xT_all[:, c * NSH:(c + 1) * NSH]),
            "W1e": W1e, "W2e": W2ex, "b1r": b1r, "b2r": b2r,
            "idx": np.ascontiguousarray(plan["idx"][c]),
        })
    import time as _time
    res = run_bass_kernel_spmd(nc, in_maps, core_ids=list(range(NCORES)))
    # repeat executions for a device-time estimate (includes PJRT dispatch
    # + host<->device transfer; NTFF profiling unavailable in this env)
    ts = []
    for _ in range(3):
        _t0 = _time.perf_counter()
        res = run_bass_kernel_spmd(nc, in_maps, core_ids=list(range(NCORES)))
        ts.append(_time.perf_counter() - _t0)
    global _LAST_EXEC_NS
    _LAST_EXEC_NS = int(min(ts) * 1e9)

    out_full = np.zeros((N, NC_), np.float32)
    nid = newid
    core = nid // NSH
    rem = nid % NSH
    tt, ll = rem // 128, rem % 128
    for c in range(NCORES):
        m = core == c
        dev = res.results[c]["out"]                    # [49, 128, 40]
        out_full[np.where(m)[0]] = dev[tt[m], ll[m]]
    # un-permute columns (device col j holds class sig[j])
    inv = np.empty(NC_, np.int64)
    inv[sig] = np.arange(NC_)
    out_full = out_full[:, inv]
    return out_full


_LAST_EXEC_NS = None

if __name__ == "__main__":
    import pickle
    inputs = pickle.load(open("inputs.pkl", "rb"))
    outp = kernel(**{k: np.asarray(v) for k, v in inputs.items()})
    exp = np.load("expected.npy")
    rel = np.linalg.norm(outp - exp) / np.linalg.norm(exp)
    print("rel:", rel)



# revision 11
# speedup vs baseline: 1177.6043x; 1177.6043x over previous
"""GAT (2-layer, PyG-style) Trainium2 Bass kernel, 8 NeuronCores.

Strategy (dst-sharded, slot-major, single-phase pair-row gather):
- Nodes ranked by in-degree, tiled into 392 global tiles of 128 lanes;
  core(g)=g%8, tile(g)=g//8 -> each core owns 49 dst tiles (6272 slots,
  50176 total rows incl. 176 fakes). newid = core*6272 + tile*128 + lane.
- Gathers are descriptor-rate-bound (~7ns/desc), so the table packs TWO
  node rows per gather row to keep indices < 32768 (signed-int16 DMA
  gather limit) without splitting edges into two padded passes:
  pair-row p holds nodes v=p and w=p+25088 contiguously.
    table1 row (512B pitch): [f_v(64)|as_v(8)|f_w(64)|as_w(8)] bf16
    table2 row (256B pitch): [f_v(40)|as_v(1)|f_w(40)|as_w(1)] bf16
  A bf16 mask stream m[lane,round] in {0,1} selects the half on-chip:
  row = A + m*(B-A), 3 DVE ops per chunk.
- Edges processed slot-major: round r of tile t gathers the r-th in-edge
  pair-row for each of the 128 dst lanes (dma_gather). Pads point at the
  fake pair-row 6250 (half 0) whose alpha_s = -3e4 => gate
  exp(leakyrelu(...)) == 0 exactly.
- Aggregation: per round, DVE scales gathered feats by g = exp(lrelu(
  alpha_s[src]+alpha_d[dst])) (c-major broadcast keeps DVE 2x mode), and
  an identity-weight matmul accumulates [g*f | g] into the tile's PSUM:
  numerator and softmax denominator in one pass.
- Between convs: each core builds its table2 shard (transpose + matmul
  with W2ext), patches its fakes' alpha_s2, AllGathers the tight shard,
  and repacks into the pair layout.
"""

import numpy as np
import ml_dtypes

import concourse.bass as bass
import concourse.bacc as bacc
import concourse.mybir as mybir
from concourse.tile import TileContext
from concourse.masks import make_identity

bf16 = ml_dtypes.bfloat16
FP = mybir.dt.float32
BF = mybir.dt.bfloat16
I16 = mybir.dt.int16

N = 50000
E = 1_600_000
F_IN = 128
H, C1 = 8, 8
D1 = 64
NC_ = 40                 # num classes
NEG = 0.2
NCORES = 8
NTILES = 49
NSH = NTILES * 128       # 6272
NTOT = NCORES * NSH      # 50176
NPAIR = NTOT // 2        # 25088 pair rows
ROW1 = 72                # node row conv1 (feats 64 + alpha_s 8)
ROW2 = 41                # conv2 (feats 40 + alpha_s 1)
P1 = 256                 # table1 pair-row pitch in bf16 elements (512B)
P2 = 128                 # table2 pair-row pitch in bf16 elements (256B)
ANEG = -30000.0
MAXG = 8192              # max idxs per dma_gather (64 rounds)
PAD = 6250               # fake pair-row (half 0) used for padding slots


# --------------------------------------------------------------------------
# host planning
# --------------------------------------------------------------------------

def _plan(edge_index):
    src = np.asarray(edge_index[0], np.int64)
    dst = np.asarray(edge_index[1], np.int64)
    loops = np.arange(N, dtype=np.int64)
    src = np.concatenate([src, loops])
    dst = np.concatenate([dst, loops])

    indeg = np.bincount(dst, minlength=N)
    order = np.argsort(-indeg, kind="stable")          # rank -> node
    ranks = np.arange(NTOT)
    g = ranks // 128
    newid_of_rank = (g % NCORES) * NSH + (g // NCORES) * 128 + ranks % 128
    newid = np.empty(N, np.int64)
    newid[order] = newid_of_rank[:N]

    # make row 6250 a fake (PAD row): move its real node to another fake
    r0 = int(np.where(newid == PAD)[0][0]) if (newid == PAD).any() else -1
    if r0 >= 0:
        newid[r0] = 50048
    e_src_row = newid[src]
    e_dst_new = newid[dst]
    e_core = e_dst_new // NSH
    e_rem = e_dst_new % NSH
    e_t = e_rem // 128
    e_lane = e_rem % 128

    # per (core, tile, lane) in-edge counts -> common per-tile round count
    flat_lane = (e_core * NTILES + e_t) * 128 + e_lane
    cnt = np.bincount(flat_lane, minlength=NCORES * NTILES * 128)
    cnt = cnt.reshape(NCORES, NTILES, 128)
    K = np.maximum(cnt.max(axis=(0, 2)), 1).astype(np.int64)
    base = np.concatenate([[0], np.cumsum(K)])
    R = int(base[-1])

    # slot assignment
    slot_p = np.full((NCORES, 128, R), PAD, np.int32)      # pair-row idx
    slot_m = np.zeros((NCORES, 128, R), np.int8)           # half bit

    key = flat_lane
    sidx = np.argsort(key, kind="stable")
    ks = key[sidx]
    newgrp = np.ones(len(ks), bool)
    newgrp[1:] = ks[1:] != ks[:-1]
    pos = np.arange(len(ks))
    start = np.maximum.accumulate(np.where(newgrp, pos, 0))
    cum = pos - start
    slot = np.empty(len(ks), np.int64)
    slot[sidx] = cum

    slot_p[e_core, e_lane, base[e_t] + slot] = e_src_row % NPAIR
    slot_m[e_core, e_lane, base[e_t] + slot] = e_src_row // NPAIR

    # chunks: split the round stream at MAXG//128-round boundaries
    CR = MAXG // 128
    chunks = []   # (r0, nr, segments=[(tile, seg_r0, seg_nr, tile_r0, done)])
    r = 0
    while r < R:
        nr = min(CR, R - r)
        segs = []
        for t in range(NTILES):
            s0, s1 = int(base[t]), int(base[t + 1])
            a, b = max(s0, r), min(s1, r + nr)
            if a < b:
                segs.append((t, a, b - a, a - s0, b == s1))
        chunks.append((r, nr, segs))
        r += nr

    # idx stream int16 [NCORES, 128, NW]: per chunk block of nr*8 cols
    blocks = []
    for (r0, nr, _) in chunks:
        blk = slot_p[:, :, r0:r0 + nr]                 # [8, 128, nr]
        flat = blk.transpose(0, 2, 1).reshape(NCORES, nr * 128)
        cols = nr * 8
        w = np.zeros((NCORES, 16, cols), np.int16)
        ii = np.arange(nr * 128)
        w[:, ii % 16, ii // 16] = flat
        blocks.append(np.tile(w, (1, 8, 1)))           # replicate to 128 parts
    idx = np.concatenate(blocks, axis=2)               # [8, 128, NW]

    msk = np.ascontiguousarray(slot_m).astype(bf16)    # [8, 128, R]

    return dict(order=order, newid=newid, K=K, chunks=chunks, idx=idx,
                msk=msk, R=R)


# --------------------------------------------------------------------------
# gather instruction (tight rows on a 256B-multiple pitch)
# --------------------------------------------------------------------------

def _gather(eng, out_ap, in_ap, idxs_ap, num_idxs, elem_size, elem_step,
            queue_num=0):
    dts = mybir.dt.size(in_ap.dtype)
    sb = elem_step * dts
    assert sb % 256 == 0 and sb // 256 < 256
    _in = eng.lower_ap_dma(in_ap, for_custom_bir_dma=True)
    return eng.add_instruction(
        mybir.InstDMAGatherAnt(
            name=eng.bass.get_next_instruction_name(),
            ins=[*_in, eng.lower_ap(idxs_ap),
                 eng.lower_val_access(eng.to_reg(num_idxs))],
            outs=[eng.lower_ap(out_ap)],
            transpose=False, num_idxs=num_idxs, elem_size=elem_size,
            stride_bytes_256=sb // 256, gen_mode=0, single_packet=False,
            queue_num=queue_num, sbuf_tokens_per_rank=0, sbuf_free_dim_per_rank=0,
            sbuf_free_dim_pad_per_rank=0, sbuf_byte_offset=0,
        ))


def _bc(ap, dims):
    """Hand-built broadcast AP: dims = list of [step, count]."""
    return bass.AP(ap.tensor, ap.offset, dims)


def _pair3(handle, j0, nchunk, width, pitch):
    """DRAM AP [p=128, a=nchunk, e=width] at the pair position of node
    row j0 + a*128 + p (all in the same half; j0 % NPAIR + nchunk*128
    must not cross the half boundary)."""
    half = j0 // NPAIR
    p0 = j0 % NPAIR
    off = half * width
    ap = handle[:]
    return bass.AP(ap.tensor, p0 * pitch + off,
                   [[pitch, 128], [128 * pitch, nchunk], [1, width]])


# --------------------------------------------------------------------------
# device program
# --------------------------------------------------------------------------

def _build(plan):
    chunks = plan["chunks"]
    NW = plan["idx"].shape[2]
    R = plan["R"]

    nc = bacc.Bacc("TRN2", num_devices=NCORES, num_swdge_queues=2)
    AF = mybir.ActivationFunctionType

    xT = nc.dram_tensor("xT", [F_IN, NTOT], BF, kind="ExternalInput")
    xTo = nc.dram_tensor("xTo", [F_IN, NSH], BF, kind="ExternalInput")
    W1e = nc.dram_tensor("W1e", [F_IN, 80], BF, kind="ExternalInput")
    W2e = nc.dram_tensor("W2e", [D1, 42], BF, kind="ExternalInput")
    b1r = nc.dram_tensor("b1r", [128, D1], FP, kind="ExternalInput")
    b2r = nc.dram_tensor("b2r", [128, NC_], FP, kind="ExternalInput")
    idx = nc.dram_tensor("idx", [128, NW], I16, kind="ExternalInput")
    mskd = nc.dram_tensor("mskd", [128, R], BF, kind="ExternalInput")
    out = nc.dram_tensor("out", [NTILES, 128, NC_], FP, kind="ExternalOutput")

    tab1 = nc.dram_tensor("tab1", [NPAIR, P1], BF, kind="Internal")
    shard2 = nc.dram_tensor("shard2", [NSH, 42], BF, kind="Internal")
    tab2t = nc.dram_tensor("tab2t", [NTOT, 42], BF, kind="Internal",
                           addr_space="Shared")
    tab2 = nc.dram_tensor("tab2", [NPAIR, P2], BF, kind="Internal")

    with TileContext(nc, num_cores=NCORES) as tc:
        with (
            tc.tile_pool(name="const", bufs=1) as const,
            tc.tile_pool(name="io", bufs=3) as io,
            tc.tile_pool(name="gbuf", bufs=2) as gbuf,
            tc.tile_pool(name="work", bufs=4) as work,
            tc.tile_pool(name="ps_acc", bufs=3, space="PSUM") as ps_acc,
            tc.tile_pool(name="ps_b", bufs=2, space="PSUM") as ps_b,
            tc.tile_pool(name="ps_e", bufs=1, space="PSUM") as ps_e,
        ):
            idf = const.tile([128, 128], FP, name="idf")
            make_identity(nc, idf[:])
            idb = const.tile([128, 128], BF, name="idb")
            nc.vector.tensor_copy(out=idb[:], in_=idf[:])
            w1 = const.tile([F_IN, 80], BF, name="w1")
            nc.sync.dma_start(out=w1[:], in_=W1e[:])
            w2 = const.tile([D1, 42], BF, name="w2")
            nc.sync.dma_start(out=w2[:], in_=W2e[:])
            b1t = const.tile([128, D1], FP, name="b1t")
            nc.sync.dma_start(out=b1t[:], in_=b1r[:])
            b2t = const.tile([128, NC_], FP, name="b2t")
            nc.sync.dma_start(out=b2t[:], in_=b2r[:])
            negt = const.tile([128, 1], BF, name="negt")
            nc.gpsimd.memset(negt[:], ANEG)
            idx_t = const.tile([128, NW], I16, name="idx_t")
            nc.sync.dma_start(out=idx_t[:], in_=idx[:])
            msk_t = const.tile([128, R], BF, name="msk_t")
            nc.sync.dma_start(out=msk_t[:], in_=mskd[:])
            ad1 = const.tile([128, NTILES * 8], FP, name="ad1")
            ad2 = const.tile([128, NTILES], FP, name="ad2")

            # ---- phase 1: replicated table1 build -------------------------
            XB = 512
            for j0 in range(0, NTOT, XB):
                xt = io.tile([128, XB], BF, tag="xt", name="xt")
                nc.sync.dma_start(out=xt[:], in_=xT[:, j0:j0 + XB])
                pb = ps_b.tile([128, 4 * ROW1], FP, tag="pb", name="pb")
                st = io.tile([128, 4 * ROW1], BF, tag="st", name="st")
                for k in range(4):
                    nc.tensor.matmul(
                        out=pb[:, k * ROW1:(k + 1) * ROW1],
                        lhsT=xt[:, k * 128:(k + 1) * 128],
                        rhs=w1[:, :ROW1], start=True, stop=True)
                eng = nc.vector if (j0 // XB) % 2 == 0 else nc.scalar
                if eng is nc.vector:
                    eng.tensor_copy(out=st[:], in_=pb[:])
                else:
                    eng.activation(st[:], pb[:], AF.Copy)
                nc.sync.dma_start(
                    out=_pair3(tab1, j0, 4, ROW1, P1), in_=st[:])

            # ---- phase 1b: own alpha_d1 ----------------------------------
            for t in range(NTILES):
                xo = io.tile([128, 128], BF, tag="xo", name="xo")
                nc.sync.dma_start(out=xo[:], in_=xTo[:, t * 128:(t + 1) * 128])
                pa = ps_e.tile([128, 8], FP, tag="pa", name="pa")
                nc.tensor.matmul(out=pa[:], lhsT=xo[:], rhs=w1[:, 72:80],
                                 start=True, stop=True)
                nc.vector.tensor_copy(out=ad1[:, t * 8:(t + 1) * 8], in_=pa[:])

            # ---- conv passes (single phase each) -------------------------
            def conv_pass(conv, tab):
                ROW = ROW1 if conv == 1 else ROW2
                PITCH = P1 if conv == 1 else P2
                psums = {}
                for ci, (r0, nr, segs) in enumerate(chunks):
                    nidx = nr * 128
                    cw = nr * 8
                    buf = gbuf.tile([128, nr, 2 * ROW], BF, tag=f"g{conv}",
                                    name=f"buf{conv}")
                    _gather(nc.gpsimd, buf[:], tab[:, :2 * ROW],
                            idx_t[:, r0 * 8: r0 * 8 + cw],
                            nidx, 2 * ROW, PITCH, queue_num=ci % 2)
                    # blend halves: A += m*(B-A), in place on the A half
                    bb = buf[:]
                    bA = _bc(bb, [bb.ap[0], [2 * ROW, nr], [1, ROW]])
                    bB = bass.AP(bb.tensor, bb.offset + ROW,
                                 [bb.ap[0], [2 * ROW, nr], [1, ROW]])
                    mv = msk_t[:, r0:r0 + nr]
                    mb = _bc(mv, [mv.ap[0], [1, nr], [0, ROW]])
                    nc.vector.tensor_tensor(out=bB, in0=bB, in1=bA,
                                            op=mybir.AluOpType.subtract)
                    nc.vector.tensor_tensor(out=bB, in0=bB, in1=mb,
                                            op=mybir.AluOpType.mult)
                    nc.vector.tensor_tensor(out=bA, in0=bA, in1=bB,
                                            op=mybir.AluOpType.add)
                    # e = alpha_s + alpha_d per segment; prelu+exp chunk-wide
                    if conv == 1:
                        e = work.tile([128, nr, 8], FP, tag="e1", name="e1")
                        gg = work.tile([128, nr, 8], BF, tag="gg1", name="gg1")
                        for (t, a, n, tr0, _) in segs:
                            o = a - r0
                            adv = ad1[:, t * 8:t * 8 + 8]
                            nc.vector.tensor_tensor(
                                out=e[:, o:o + n, :],
                                in0=bass.AP(bb.tensor,
                                            bb.offset + o * 2 * ROW + 64,
                                            [bb.ap[0], [2 * ROW, n], [1, 8]]),
                                in1=_bc(adv[:], [adv[:].ap[0], [0, n], [1, 8]]),
                                op=mybir.AluOpType.add)
                        es = work.tile([128, nr, 8], FP, tag="es1", name="es1")
                        nc.vector.tensor_scalar(es[:], e[:], NEG, None,
                                                mybir.AluOpType.mult)
                        nc.vector.tensor_tensor(out=e[:], in0=e[:], in1=es[:],
                                                op=mybir.AluOpType.max)
                        nc.scalar.activation(gg[:], e[:], AF.Exp)
                        gb = gg[:]
                        b4 = _bc(bb, [bb.ap[0], [2 * ROW, nr], [8, 8], [1, 8]])
                        nc.vector.tensor_tensor(
                            out=b4, in0=b4,
                            in1=_bc(gb, [gb.ap[0], [8, nr], [0, 8], [1, 8]]),
                            op=mybir.AluOpType.mult)
                        nc.vector.tensor_copy(
                            out=bass.AP(bb.tensor, bb.offset + 64,
                                        [bb.ap[0], [2 * ROW, nr], [1, 8]]),
                            in_=gg[:])
                    else:
                        e = work.tile([128, nr, 1], FP, tag="e2", name="e2")
                        gg = work.tile([128, nr, 1], BF, tag="gg2", name="gg2")
                        g8 = work.tile([128, nr, 8], BF, tag="g8", name="g8")
                        for (t, a, n, tr0, _) in segs:
                            o = a - r0
                            adv = ad2[:, t:t + 1]
                            nc.vector.tensor_tensor(
                                out=e[:, o:o + n, :],
                                in0=bass.AP(bb.tensor,
                                            bb.offset + o * 2 * ROW + 40,
                                            [bb.ap[0], [2 * ROW, n], [1, 1]]),
                                in1=_bc(adv[:], [adv[:].ap[0], [0, n], [0, 1]]),
                                op=mybir.AluOpType.add)
                        es = work.tile([128, nr, 1], FP, tag="es2", name="es2")
                        nc.vector.tensor_scalar(es[:], e[:], NEG, None,
                                                mybir.AluOpType.mult)
                        nc.vector.tensor_tensor(out=e[:], in0=e[:], in1=es[:],
                                                op=mybir.AluOpType.max)
                        nc.scalar.activation(gg[:], e[:], AF.Exp)
                        gb = gg[:]
                        nc.vector.tensor_copy(
                            out=g8[:],
                            in_=_bc(gb, [gb.ap[0], [1, nr], [0, 8]]))
                        g8b = g8[:]
                        b4 = _bc(bb, [bb.ap[0], [2 * ROW, nr], [8, 5], [1, 8]])
                        nc.vector.tensor_tensor(
                            out=b4, in0=b4,
                            in1=_bc(g8b, [g8b.ap[0], [8, nr], [0, 5], [1, 8]]),
                            op=mybir.AluOpType.mult)
                        nc.vector.tensor_copy(
                            out=bass.AP(bb.tensor, bb.offset + 40,
                                        [bb.ap[0], [2 * ROW, nr], [1, 1]]),
                            in_=gg[:])
                    # accumulate rounds into per-tile psum
                    for (t, a, n, tr0, done) in segs:
                        if t not in psums:
                            psums[t] = ps_acc.tile([128, ROW1], FP, tag="acc",
                                                   name=f"ps{conv}_{t}")
                        pt = psums[t]
                        for r_ in range(n):
                            nc.tensor.matmul(
                                out=pt[:, :ROW], lhsT=idb[:, :128],
                                rhs=bass.AP(bb.tensor,
                                            bb.offset + (a - r0 + r_) * 2 * ROW,
                                            [bb.ap[0], [1, ROW]]),
                                start=(tr0 + r_ == 0), stop=done and r_ == n - 1)
                        if done:
                            yield t, pt
                            del psums[t]

            for t, pt in conv_pass(1, tab1):
                nd = work.tile([128, ROW1], FP, tag="nd1", name="nd1")
                nc.vector.tensor_copy(out=nd[:], in_=pt[:, :ROW1])
                den = work.tile([128, 8], FP, tag="den1", name="den1")
                nc.vector.tensor_scalar(den[:], nd[:, 64:72], 1e-16, None,
                                        mybir.AluOpType.max)
                rec = work.tile([128, 8], FP, tag="rec1", name="rec1")
                nc.vector.reciprocal(rec[:], den[:])
                h1 = work.tile([128, D1], FP, tag="h1", name="h1")
                rb = rec[:]
                h1v = h1[:]
                ndv = nd[:]
                nc.vector.tensor_tensor(
                    out=_bc(h1v, [h1v.ap[0], [8, 8], [1, 8]]),
                    in0=_bc(ndv, [ndv.ap[0], [8, 8], [1, 8]]),
                    in1=_bc(rb, [rb.ap[0], [0, 8], [1, 8]]),
                    op=mybir.AluOpType.mult)
                nc.vector.tensor_tensor(out=h1[:], in0=h1[:], in1=b1t[:],
                                        op=mybir.AluOpType.add)
                nc.vector.tensor_scalar(h1[:], h1[:], 0.0, None,
                                        mybir.AluOpType.max)
                ptr = ps_e.tile([64, 128], FP, tag="tr", name="ptr")
                nc.tensor.transpose(out=ptr[:], in_=h1[:], identity=idf[:])
                h1T = work.tile([64, 128], BF, tag="h1T", name="h1T")
                nc.vector.tensor_copy(out=h1T[:], in_=ptr[:])
                pf2 = ps_e.tile([128, 42], FP, tag="pf2", name="pf2")
                nc.tensor.matmul(out=pf2[:], lhsT=h1T[:], rhs=w2[:],
                                 start=True, stop=True)
                nc.vector.tensor_copy(out=ad2[:, t:t + 1], in_=pf2[:, 41:42])
                st2 = work.tile([128, 42], BF, tag="st2", name="st2")
                nc.vector.tensor_copy(out=st2[:], in_=pf2[:])
                nc.sync.dma_start(out=shard2[t * 128:(t + 1) * 128, :],
                                  in_=st2[:])

            # allgather, repack into the pair layout
            nc.gpsimd.collective_compute(
                "AllGather", mybir.AluOpType.bypass,
                replica_groups=[list(range(NCORES))],
                ins=[shard2[:]], outs=[tab2t[:]])
            RPB = 512
            for j0 in range(0, NTOT, RPB):
                rp = io.tile([128, 4 * ROW2], BF, tag="rp", name="rp")
                nc.sync.dma_start(
                    out=rp[:],
                    in_=bass.AP(tab2t[:].tensor, j0 * 42,
                                [[42, 128], [128 * 42, 4], [1, ROW2]]))
                nc.sync.dma_start(out=_pair3(tab2, j0, 4, ROW2, P2), in_=rp[:])
            # patch all fake rows' alpha_s2 (pair positions, same all cores)
            nc.sync.dma_start(out=tab2[6250:6251, 40:41], in_=negt[:1])
            nc.sync.dma_start(out=tab2[18768:18816, 81:82], in_=negt[:48])
            nc.sync.dma_start(out=tab2[24961:25088, 81:82], in_=negt[:127])

            for t, pt in conv_pass(2, tab2):
                nd = work.tile([128, ROW2], FP, tag="nd2", name="nd2")
                nc.vector.tensor_copy(out=nd[:], in_=pt[:, :ROW2])
                den = work.tile([128, 1], FP, tag="den2", name="den2")
                nc.vector.tensor_scalar(den[:], nd[:, 40:41], 1e-16, None,
                                        mybir.AluOpType.max)
                rec = work.tile([128, 1], FP, tag="rec2", name="rec2")
                nc.vector.reciprocal(rec[:], den[:])
                o2 = work.tile([128, NC_], FP, tag="o2", name="o2")
                nc.vector.tensor_scalar(o2[:], nd[:, 0:40], rec[:, 0:1], None,
                                        mybir.AluOpType.mult)
                nc.vector.tensor_tensor(out=o2[:], in0=o2[:], in1=b2t[:],
                                        op=mybir.AluOpType.add)
                mx = work.tile([128, 1], FP, tag="mx", name="mx")
                nc.vector.tensor_reduce(mx[:], o2[:], mybir.AxisListType.X,
                                        mybir.AluOpType.max)
                nc.vector.tensor_scalar(o2[:], o2[:], mx[:, 0:1], None,
                                        mybir.AluOpType.subtract)
                ex = work.tile([128, NC_], FP, tag="ex", name="ex")
                sm = work.tile([128, 1], FP, tag="sm", name="sm")
                nc.scalar.activation(ex[:], o2[:], AF.Exp, accum_out=sm[:])
                ls = work.tile([128, 1], FP, tag="ls", name="ls")
                nc.scalar.activation(ls[:], sm[:], AF.Ln)
                nc.vector.tensor_scalar(o2[:], o2[:], ls[:, 0:1], None,
                                        mybir.AluOpType.subtract)
                nc.sync.dma_start(out=out[t], in_=o2[:])

    nc.finalize()
    return nc


# --------------------------------------------------------------------------
# exec: compile once, keep inputs device-resident, chain executes so the
# axon RPC latency (~77ms round-trip regardless of kernel) pipelines away.
# HW time estimate = steady-state throughput: total wall / K chained runs.
# --------------------------------------------------------------------------

def _run_and_time(nc, in_maps, n_reps=250, n_chain=1):
    import time
    import jax
    from jax.sharding import Mesh, PartitionSpec, NamedSharding
    from jax.experimental.shard_map import shard_map
    from concourse import bass2jax

    bass2jax.install_neuronx_cc_hook()
    pname = nc.partition_id_tensor.name if nc.partition_id_tensor else None
    in_names, out_names, out_avals, zero_outs = [], [], [], []
    for alloc in nc.m.functions[0].allocations:
        if not isinstance(alloc, mybir.MemoryLocationSet):
            continue
        name = alloc.memorylocations[0].name
        if alloc.kind == "ExternalInput":
            if name != pname:
                in_names.append(name)
        elif alloc.kind == "ExternalOutput":
            out_names.append(name)
            shape = tuple(alloc.tensor_shape)
            dt = mybir.dt.np(alloc.dtype)
            out_avals.append(jax.core.ShapedArray(shape, dt))
            zero_outs.append(np.zeros(shape, dt))
    n_params = len(in_names)
    all_in = in_names + out_names + ([pname] if pname else [])

    def _body(*args):
        # n_chain back-to-back executions in one XLA program (fori_loop with
        # the bind as the body): each iteration's outputs become the next
        # iteration's scratch output buffers, so a single host dispatch runs
        # the kernel n_chain times device-side with zero host involvement.
        ins = list(args[:n_params])
        outs = tuple(args[n_params:])
        pid = [bass2jax.partition_id_tensor()] if pname is not None else []

        def it(_, carry):
            return tuple(bass2jax._bass_exec_p.bind(
                *ins, *carry, *pid,
                out_avals=tuple(out_avals), in_names=tuple(all_in),
                out_names=tuple(out_names), lowering_input_output_aliases=(),
                sim_require_finite=True, sim_require_nnan=True, nc=nc))

        if n_chain == 1:
            return it(0, outs)
        return jax.lax.fori_loop(0, n_chain, it, outs)

    devices = jax.devices()[:NCORES]
    mesh = Mesh(np.asarray(devices), ("core",))
    nin = n_params + len(out_names)
    f = jax.jit(
        shard_map(_body, mesh=mesh, in_specs=(PartitionSpec("core"),) * nin,
                  out_specs=(PartitionSpec("core"),) * len(out_names),
                  check_rep=False),
        donate_argnums=tuple(range(n_params, nin)), keep_unused=True)
    sh = NamedSharding(mesh, PartitionSpec("core"))
    dev_in = [jax.device_put(
        np.concatenate([m[nm] for m in in_maps], axis=0), sh)
        for nm in in_names]
    zput = lambda: [jax.device_put(
        np.zeros((NCORES * z.shape[0], *z.shape[1:]), z.dtype), sh)
        for z in zero_outs]

    outs = f(*dev_in, *zput())            # compile + warm-up; keep results
    jax.block_until_ready(outs)
    res = [np.asarray(o) for o in outs]
    results = [{nm: res[i].reshape(NCORES, *out_avals[i].shape)[c]
                for i, nm in enumerate(out_names)} for c in range(NCORES)]

    # timed: K chained dispatches of n_chain executions each, donating the
    # previous outputs as scratch
    best = None
    for _ in range(3):
        cur = f(*dev_in, *zput())
        jax.block_until_ready(cur)
        t0 = time.perf_counter()
        for _ in range(n_reps):
            cur = f(*dev_in, *cur)
        jax.block_until_ready(cur)
        dt = (time.perf_counter() - t0) / (n_reps * n_chain)
        best = dt if best is None else min(best, dt)
    return results, int(best * 1e9)


# --------------------------------------------------------------------------
# host entry
# --------------------------------------------------------------------------

def kernel(x, edge_index, W1, as1, ad1, b1, W2, as2, ad2, b2):
    x = np.asarray(x, np.float32)
    ei = np.asarray(edge_index)
    W1 = np.asarray(W1, np.float32); as1 = np.asarray(as1, np.float32)
    ad1 = np.asarray(ad1, np.float32); b1 = np.asarray(b1, np.float32)
    W2 = np.asarray(W2, np.float32); as2 = np.asarray(as2, np.float32)
    ad2 = np.asarray(ad2, np.float32); b2 = np.asarray(b2, np.float32)

    plan = _plan(ei)
    newid, order = plan["newid"], plan["order"]

    # W1ext: [128, 80] = [W1 c-major | W1@as1_h | W1@ad1_h]
    W1cm = W1.reshape(F_IN, H, C1).transpose(0, 2, 1).reshape(F_IN, D1)
    Was = np.stack([W1[:, h * C1:(h + 1) * C1] @ as1[h] for h in range(H)], 1)
    Wad = np.stack([W1[:, h * C1:(h + 1) * C1] @ ad1[h] for h in range(H)], 1)
    W1e = np.concatenate([W1cm, Was, Wad], axis=1).astype(bf16)

    # x_fake: alpha_s1 = ANEG (all heads), alpha_d1 = 0
    M = np.concatenate([Was, Wad], axis=1)                    # [128, 16]
    rhs = np.concatenate([np.full(8, ANEG), np.zeros(8)])
    v = M @ np.linalg.solve(M.T @ M, rhs)
    xT_all = np.zeros((F_IN, NTOT), np.float32)
    xT_all[:, newid] = x.T
    fake = np.setdiff1d(np.arange(NTOT), newid)
    xT_all[:, fake] = v[:, None]
    xT_all = xT_all.astype(bf16)

    # conv2 fake-head col permutation: orig col o = h*5+c ; new j = c*8+h
    sig = np.empty(NC_, np.int64)
    for hh in range(8):
        for cc in range(5):
            sig[cc * 8 + hh] = hh * 5 + cc
    W2p = W2[:, sig]
    W2ex = np.concatenate([W2p, W2 @ as2[0][:, None], W2 @ ad2[0][:, None]],
                          axis=1)                             # [64, 42]
    # h1 columns are c-major (c*8+h); permute W2ext rows to match
    rowperm = np.empty(D1, np.int64)
    for hh in range(H):
        for cc in range(C1):
            rowperm[cc * 8 + hh] = hh * C1 + cc
    W2ex = W2ex[rowperm].astype(bf16)

    b1cm = b1.reshape(H, C1).T.reshape(D1)
    b1r = np.tile(b1cm, (128, 1)).astype(np.float32)
    b2r = np.tile(b2[sig], (128, 1)).astype(np.float32)

    nc = _build(plan)
    in_maps = []
    for c in range(NCORES):
        in_maps.append({
            "xT": xT_all,
            "xTo": np.ascontiguousarray(xT_all[:, c * NSH:(c + 1) * NSH]),
            "W1e": W1e, "W2e": W2ex, "b1r": b1r, "b2r": b2r,
            "idx": np.ascontiguousarray(plan["idx"][c]),
            "mskd": np.ascontiguousarray(plan["msk"][c]),
        })
    results, exec_ns = _run_and_time(nc, in_maps)
    global _LAST_EXEC_NS
    _LAST_EXEC_NS = exec_ns

    out_full = np.zeros((N, NC_), np.float32)
    nid = newid
    core = nid // NSH
    rem = nid % NSH
    tt, ll = rem // 128, rem % 128
    for c in range(NCORES):
        m = core == c
        dev = results[c]["out"]                        # [49, 128, 40]
        out_full[np.where(m)[0]] = dev[tt[m], ll[m]]
    # un-permute columns (device col j holds class sig[j])
    inv = np.empty(NC_, np.int64)
    inv[sig] = np.arange(NC_)
    out_full = out_full[:, inv]
    return out_full


_LAST_EXEC_NS = None

if __name__ == "__main__":
    import pickle
    inputs = pickle.load(open("inputs.pkl", "rb"))
    outp = kernel(**{k: np.asarray(v) for k, v in inputs.items()})
    exp = np.load("expected.npy")
    rel = np.linalg.norm(outp - exp) / np.linalg.norm(exp)
    print("rel:", rel)


# revision 13
# speedup vs baseline: 1365.0807x; 1.1592x over previous
"""GAT (2-layer, PyG-style) Trainium2 Bass kernel, 8 NeuronCores.

Strategy (dst-sharded, slot-major, single-phase pair-row gather):
- Nodes ranked by in-degree, tiled into 392 global tiles of 128 lanes;
  core(g)=g%8, tile(g)=g//8 -> each core owns 49 dst tiles (6272 slots,
  50176 total rows incl. 176 fakes). newid = core*6272 + tile*128 + lane.
- Gathers are descriptor-rate-bound (~7ns/desc), so the table packs TWO
  node rows per gather row to keep indices < 32768 (signed-int16 DMA
  gather limit) without splitting edges into two padded passes:
  pair-row p holds nodes v=p and w=p+25088 contiguously.
    table1 row (512B pitch): [f_v(64)|as_v(8)|f_w(64)|as_w(8)] bf16
    table2 row (256B pitch): [f_v(40)|as_v(1)|f_w(40)|as_w(1)] bf16
  A bf16 mask stream m[lane,round] in {0,1} selects the half on-chip:
  row = A + m*(B-A), 3 DVE ops per chunk.
- Edges processed slot-major: round r of tile t gathers the r-th in-edge
  pair-row for each of the 128 dst lanes (dma_gather). Pads point at the
  fake pair-row 6250 (half 0) whose alpha_s = -3e4 => gate
  exp(leakyrelu(...)) == 0 exactly.
- Aggregation: per round, DVE scales gathered feats by g = exp(lrelu(
  alpha_s[src]+alpha_d[dst])) (c-major broadcast keeps DVE 2x mode), and
  an identity-weight matmul accumulates [g*f | g] into the tile's PSUM:
  numerator and softmax denominator in one pass.
- Between convs: each core builds its table2 shard (transpose + matmul
  with W2ext) directly in padded pair-row form; every 7 finished tiles
  are AllGathered into their contiguous block of the piece-major conv2
  table (no repack), then the fakes' alpha_s2 rows are patched.
"""

import numpy as np
import ml_dtypes

import concourse.bass as bass
import concourse.bacc as bacc
import concourse.mybir as mybir
from concourse.tile import TileContext
from concourse.masks import make_identity

bf16 = ml_dtypes.bfloat16
FP = mybir.dt.float32
BF = mybir.dt.bfloat16
I16 = mybir.dt.int16

N = 50000
E = 1_600_000
F_IN = 128
H, C1 = 8, 8
D1 = 64
NC_ = 40                 # num classes
NEG = 0.2
NCORES = 8
NTILES = 49
NSH = NTILES * 128       # 6272
NTOT = NCORES * NSH      # 50176
NPAIR = NTOT // 2        # 25088 pair rows
ROW1 = 72                # node row conv1 (feats 64 + alpha_s 8)
ROW2 = 41                # conv2 (feats 40 + alpha_s 1)
P1 = 256                 # table1 pair-row pitch in bf16 elements (512B)
P2 = 128                 # table2 pair-row pitch in bf16 elements (256B)
ANEG = -30000.0
MAXG = 8192              # max idxs per dma_gather (64 rounds)
PAD = 6250               # fake pair-row (half 0) used for padding slots


# --------------------------------------------------------------------------
# host planning
# --------------------------------------------------------------------------

def _plan(edge_index):
    src = np.asarray(edge_index[0], np.int64)
    dst = np.asarray(edge_index[1], np.int64)
    loops = np.arange(N, dtype=np.int64)
    src = np.concatenate([src, loops])
    dst = np.concatenate([dst, loops])

    indeg = np.bincount(dst, minlength=N)
    order = np.argsort(-indeg, kind="stable")          # rank -> node
    ranks = np.arange(NTOT)
    g = ranks // 128
    newid_of_rank = (g % NCORES) * NSH + (g // NCORES) * 128 + ranks % 128
    newid = np.empty(N, np.int64)
    newid[order] = newid_of_rank[:N]

    # make row 6250 a fake (PAD row): move its real node to another fake
    r0 = int(np.where(newid == PAD)[0][0]) if (newid == PAD).any() else -1
    if r0 >= 0:
        newid[r0] = 50048
    e_src_row = newid[src]
    e_dst_new = newid[dst]
    e_core = e_dst_new // NSH
    e_rem = e_dst_new % NSH
    e_t = e_rem // 128
    e_lane = e_rem % 128

    # per (core, tile, lane) in-edge counts -> common per-tile round count
    flat_lane = (e_core * NTILES + e_t) * 128 + e_lane
    cnt = np.bincount(flat_lane, minlength=NCORES * NTILES * 128)
    cnt = cnt.reshape(NCORES, NTILES, 128)
    K = np.maximum(cnt.max(axis=(0, 2)), 1).astype(np.int64)
    base = np.concatenate([[0], np.cumsum(K)])
    R = int(base[-1])

    # slot assignment. conv1 pair-row p1=(v>>7)*64+(v&63); conv2 pair-rows
    # are piece-major (7 tiles x 8 cores per piece) so each overlapped
    # AllGather piece lands in a contiguous block of the final table:
    # p2=(t//7)*3584 + core*448 + (t%7)*64 + (v&63). half=(v>>6)&1 in both.
    def p2_of(v):
        c = v // NSH
        t = (v % NSH) // 128
        return (t // 7) * 3584 + c * 448 + (t % 7) * 64 + (v & 63)
    slot_p = np.full((NCORES, 128, R), (PAD >> 7) * 64 + (PAD & 63), np.int32)
    slot_q = np.full((NCORES, 128, R), p2_of(PAD), np.int32)
    slot_m = np.full((NCORES, 128, R), (PAD >> 6) & 1, np.int8)

    key = flat_lane
    sidx = np.argsort(key, kind="stable")
    ks = key[sidx]
    newgrp = np.ones(len(ks), bool)
    newgrp[1:] = ks[1:] != ks[:-1]
    pos = np.arange(len(ks))
    start = np.maximum.accumulate(np.where(newgrp, pos, 0))
    cum = pos - start
    slot = np.empty(len(ks), np.int64)
    slot[sidx] = cum

    slot_p[e_core, e_lane, base[e_t] + slot] = \
        (e_src_row >> 7) * 64 + (e_src_row & 63)
    slot_q[e_core, e_lane, base[e_t] + slot] = p2_of(e_src_row)
    slot_m[e_core, e_lane, base[e_t] + slot] = (e_src_row >> 6) & 1

    # chunks: split the round stream at MAXG//128-round boundaries
    CR = MAXG // 128
    chunks = []   # (r0, nr, segments=[(tile, seg_r0, seg_nr, tile_r0, done)])
    r = 0
    while r < R:
        nr = min(CR, R - r)
        segs = []
        for t in range(NTILES):
            s0, s1 = int(base[t]), int(base[t + 1])
            a, b = max(s0, r), min(s1, r + nr)
            if a < b:
                segs.append((t, a, b - a, a - s0, b == s1))
        chunks.append((r, nr, segs))
        r += nr

    # idx streams int16 [NCORES, 128, NW]: per chunk block of nr*8 cols
    def mk_idx(slots):
        blocks = []
        for (r0, nr, _) in chunks:
            blk = slots[:, :, r0:r0 + nr]              # [8, 128, nr]
            flat = blk.transpose(0, 2, 1).reshape(NCORES, nr * 128)
            cols = nr * 8
            w = np.zeros((NCORES, 16, cols), np.int16)
            ii = np.arange(nr * 128)
            w[:, ii % 16, ii // 16] = flat
            blocks.append(np.tile(w, (1, 8, 1)))       # replicate to 128 parts
        return np.concatenate(blocks, axis=2)
    idx = np.concatenate([mk_idx(slot_p), mk_idx(slot_q)], axis=2)

    msk = np.ascontiguousarray(slot_m).astype(bf16)    # [8, 128, R]

    return dict(order=order, newid=newid, K=K, chunks=chunks, idx=idx,
                msk=msk, R=R)


# --------------------------------------------------------------------------
# gather instruction (tight rows on a 256B-multiple pitch)
# --------------------------------------------------------------------------

def _gather(eng, out_ap, in_ap, idxs_ap, num_idxs, elem_size, elem_step,
            queue_num=0):
    dts = mybir.dt.size(in_ap.dtype)
    sb = elem_step * dts
    assert sb % 256 == 0 and sb // 256 < 256
    _in = eng.lower_ap_dma(in_ap, for_custom_bir_dma=True)
    return eng.add_instruction(
        mybir.InstDMAGatherAnt(
            name=eng.bass.get_next_instruction_name(),
            ins=[*_in, eng.lower_ap(idxs_ap),
                 eng.lower_val_access(eng.to_reg(num_idxs))],
            outs=[eng.lower_ap(out_ap)],
            transpose=False, num_idxs=num_idxs, elem_size=elem_size,
            stride_bytes_256=sb // 256, gen_mode=0, single_packet=False,
            queue_num=queue_num, sbuf_tokens_per_rank=0, sbuf_free_dim_per_rank=0,
            sbuf_free_dim_pad_per_rank=0, sbuf_byte_offset=0,
        ))


def _bc(ap, dims):
    """Hand-built broadcast AP: dims = list of [step, count]."""
    return bass.AP(ap.tensor, ap.offset, dims)


def _pair3(handle, j0, nchunk, width, pitch):
    """DRAM AP [p=128, a=nchunk, e=width] at the pair position of node
    row j0 + a*128 + p (all in the same half; j0 % NPAIR + nchunk*128
    must not cross the half boundary)."""
    half = j0 // NPAIR
    p0 = j0 % NPAIR
    off = half * width
    ap = handle[:]
    return bass.AP(ap.tensor, p0 * pitch + off,
                   [[pitch, 128], [128 * pitch, nchunk], [1, width]])


# --------------------------------------------------------------------------
# device program
# --------------------------------------------------------------------------

def _build(plan):
    chunks = plan["chunks"]
    NW = plan["idx"].shape[2]          # two streams: conv1 at 0, conv2 at NW2
    NW2 = NW // 2
    R = plan["R"]

    nc = bacc.Bacc("TRN2", num_devices=NCORES, num_swdge_queues=2)
    AF = mybir.ActivationFunctionType

    xT = nc.dram_tensor("xT", [F_IN, NTOT], BF, kind="ExternalInput")
    xTo = nc.dram_tensor("xTo", [F_IN, NSH], BF, kind="ExternalInput")
    W1e = nc.dram_tensor("W1e", [F_IN, 80], BF, kind="ExternalInput")
    W2e = nc.dram_tensor("W2e", [D1, 42], BF, kind="ExternalInput")
    b1r = nc.dram_tensor("b1r", [128, D1], FP, kind="ExternalInput")
    b2r = nc.dram_tensor("b2r", [128, NC_], FP, kind="ExternalInput")
    idx = nc.dram_tensor("idx", [128, NW], I16, kind="ExternalInput")
    mskd = nc.dram_tensor("mskd", [128, R], BF, kind="ExternalInput")
    out = nc.dram_tensor("out", [NTILES, 128, NC_], FP, kind="ExternalOutput")

    tab1 = nc.dram_tensor("tab1", [NPAIR, P1], BF, kind="Internal")
    shard2 = nc.dram_tensor("shard2", [NSH // 2, P2], BF, kind="Internal")
    tab2v = nc.dram_tensor("tab2v", [NPAIR, P2], BF, kind="Internal",
                           addr_space="Shared")

    with TileContext(nc, num_cores=NCORES) as tc:
        with (
            tc.tile_pool(name="const", bufs=1) as const,
            tc.tile_pool(name="io", bufs=3) as io,
            tc.tile_pool(name="gbuf", bufs=2) as gbuf,
            tc.tile_pool(name="work", bufs=4) as work,
            tc.tile_pool(name="ps_acc", bufs=3, space="PSUM") as ps_acc,
            tc.tile_pool(name="ps_b", bufs=2, space="PSUM") as ps_b,
            tc.tile_pool(name="ps_e", bufs=1, space="PSUM") as ps_e,
        ):
            idf = const.tile([128, 128], FP, name="idf")
            make_identity(nc, idf[:])
            idb = const.tile([128, 128], BF, name="idb")
            nc.vector.tensor_copy(out=idb[:], in_=idf[:])
            w1 = const.tile([F_IN, 80], BF, name="w1")
            nc.sync.dma_start(out=w1[:], in_=W1e[:])
            w2 = const.tile([D1, 42], BF, name="w2")
            nc.sync.dma_start(out=w2[:], in_=W2e[:])
            b1t = const.tile([128, D1], FP, name="b1t")
            nc.sync.dma_start(out=b1t[:], in_=b1r[:])
            b2t = const.tile([128, NC_], FP, name="b2t")
            nc.sync.dma_start(out=b2t[:], in_=b2r[:])
            negt = const.tile([128, 1], BF, name="negt")
            nc.gpsimd.memset(negt[:], ANEG)
            idx_t = const.tile([128, NW], I16, name="idx_t")
            nc.sync.dma_start(out=idx_t[:], in_=idx[:])
            msk_t = const.tile([128, R], BF, name="msk_t")
            nc.sync.dma_start(out=msk_t[:], in_=mskd[:])
            ad1 = const.tile([128, NTILES * 8], FP, name="ad1")
            ad2 = const.tile([128, NTILES], FP, name="ad2")

            # ---- phase 1: replicated table1 build -------------------------
            XB = 512
            for j0 in range(0, NTOT, XB):
                xt = io.tile([128, XB], BF, tag="xt", name="xt")
                nc.sync.dma_start(out=xt[:], in_=xT[:, j0:j0 + XB])
                pb = ps_b.tile([128, 4 * ROW1], FP, tag="pb", name="pb")
                st = io.tile([128, 4 * ROW1], BF, tag="st", name="st")
                for k in range(4):
                    nc.tensor.matmul(
                        out=pb[:, k * ROW1:(k + 1) * ROW1],
                        lhsT=xt[:, k * 128:(k + 1) * 128],
                        rhs=w1[:, :ROW1], start=True, stop=True)
                eng = nc.vector if (j0 // XB) % 2 == 0 else nc.scalar
                if eng is nc.vector:
                    eng.tensor_copy(out=st[:], in_=pb[:])
                else:
                    eng.activation(st[:], pb[:], AF.Copy)
                sv = st[:]
                pb0 = (j0 // 128) * 64
                for hf in range(2):
                    nc.sync.dma_start(
                        out=bass.AP(tab1[:].tensor, pb0 * P1 + hf * ROW1,
                                    [[P1, 64], [64 * P1, 4], [1, ROW1]]),
                        in_=bass.AP(sv.tensor,
                                    sv.offset + hf * 64 * sv.ap[0][0]
                                    if False else sv.offset,
                                    [[sv.ap[0][0], 64], [ROW1, 4], [1, ROW1]])
                        if hf == 0 else
                        bass.AP(st[64:128, :].tensor, st[64:128, :].offset,
                                [[st[64:128, :].ap[0][0], 64],
                                 [ROW1, 4], [1, ROW1]]))

            # ---- phase 1b: own alpha_d1 ----------------------------------
            for t in range(NTILES):
                xo = io.tile([128, 128], BF, tag="xo", name="xo")
                nc.sync.dma_start(out=xo[:], in_=xTo[:, t * 128:(t + 1) * 128])
                pa = ps_e.tile([128, 8], FP, tag="pa", name="pa")
                nc.tensor.matmul(out=pa[:], lhsT=xo[:], rhs=w1[:, 72:80],
                                 start=True, stop=True)
                nc.vector.tensor_copy(out=ad1[:, t * 8:(t + 1) * 8], in_=pa[:])

            # ---- conv passes (single phase each) -------------------------
            def conv_pass(conv, tab, col0):
                ROW = ROW1 if conv == 1 else ROW2
                PITCH = P1 if conv == 1 else P2
                psums = {}
                for ci, (r0, nr, segs) in enumerate(chunks):
                    nidx = nr * 128
                    cw = nr * 8
                    buf = gbuf.tile([128, nr, 2 * ROW], BF, tag=f"g{conv}",
                                    name=f"buf{conv}")
                    _gather(nc.gpsimd, buf[:], tab[:, :2 * ROW],
                            idx_t[:, col0 + r0 * 8: col0 + r0 * 8 + cw],
                            nidx, 2 * ROW, PITCH, queue_num=ci % 2)
                    # blend halves: A += m*(B-A), in place on the A half
                    bb = buf[:]
                    bA = _bc(bb, [bb.ap[0], [2 * ROW, nr], [1, ROW]])
                    bB = bass.AP(bb.tensor, bb.offset + ROW,
                                 [bb.ap[0], [2 * ROW, nr], [1, ROW]])
                    mv = msk_t[:, r0:r0 + nr]
                    mb = _bc(mv, [mv.ap[0], [1, nr], [0, ROW]])
                    nc.vector.tensor_tensor(out=bB, in0=bB, in1=bA,
                                            op=mybir.AluOpType.subtract)
                    nc.vector.tensor_tensor(out=bB, in0=bB, in1=mb,
                                            op=mybir.AluOpType.mult)
                    nc.vector.tensor_tensor(out=bA, in0=bA, in1=bB,
                                            op=mybir.AluOpType.add)
                    # e = alpha_s + alpha_d per segment; prelu+exp chunk-wide
                    if conv == 1:
                        e = work.tile([128, nr, 8], FP, tag="e1", name="e1")
                        gg = work.tile([128, nr, 8], BF, tag="gg1", name="gg1")
                        for (t, a, n, tr0, _) in segs:
                            o = a - r0
                            adv = ad1[:, t * 8:t * 8 + 8]
                            nc.vector.tensor_tensor(
                                out=e[:, o:o + n, :],
                                in0=bass.AP(bb.tensor,
                                            bb.offset + o * 2 * ROW + 64,
                                            [bb.ap[0], [2 * ROW, n], [1, 8]]),
                                in1=_bc(adv[:], [adv[:].ap[0], [0, n], [1, 8]]),
                                op=mybir.AluOpType.add)
                        es = work.tile([128, nr, 8], FP, tag="es1", name="es1")
                        nc.vector.tensor_scalar(es[:], e[:], NEG, None,
                                                mybir.AluOpType.mult)
                        nc.vector.tensor_tensor(out=e[:], in0=e[:], in1=es[:],
                                                op=mybir.AluOpType.max)
                        nc.scalar.activation(gg[:], e[:], AF.Exp)
                        gb = gg[:]
                        b4 = _bc(bb, [bb.ap[0], [2 * ROW, nr], [8, 8], [1, 8]])
                        nc.vector.tensor_tensor(
                            out=b4, in0=b4,
                            in1=_bc(gb, [gb.ap[0], [8, nr], [0, 8], [1, 8]]),
                            op=mybir.AluOpType.mult)
                        nc.vector.tensor_copy(
                            out=bass.AP(bb.tensor, bb.offset + 64,
                                        [bb.ap[0], [2 * ROW, nr], [1, 8]]),
                            in_=gg[:])
                    else:
                        e = work.tile([128, nr, 1], FP, tag="e2", name="e2")
                        gg = work.tile([128, nr, 1], BF, tag="gg2", name="gg2")
                        g8 = work.tile([128, nr, 8], BF, tag="g8", name="g8")
                        for (t, a, n, tr0, _) in segs:
                            o = a - r0
                            adv = ad2[:, t:t + 1]
                            nc.vector.tensor_tensor(
                                out=e[:, o:o + n, :],
                                in0=bass.AP(bb.tensor,
                                            bb.offset + o * 2 * ROW + 40,
                                            [bb.ap[0], [2 * ROW, n], [1, 1]]),
                                in1=_bc(adv[:], [adv[:].ap[0], [0, n], [0, 1]]),
                                op=mybir.AluOpType.add)
                        es = work.tile([128, nr, 1], FP, tag="es2", name="es2")
                        nc.vector.tensor_scalar(es[:], e[:], NEG, None,
                                                mybir.AluOpType.mult)
                        nc.vector.tensor_tensor(out=e[:], in0=e[:], in1=es[:],
                                                op=mybir.AluOpType.max)
                        nc.scalar.activation(gg[:], e[:], AF.Exp)
                        gb = gg[:]
                        nc.vector.tensor_copy(
                            out=g8[:],
                            in_=_bc(gb, [gb.ap[0], [1, nr], [0, 8]]))
                        g8b = g8[:]
                        b4 = _bc(bb, [bb.ap[0], [2 * ROW, nr], [8, 5], [1, 8]])
                        nc.vector.tensor_tensor(
                            out=b4, in0=b4,
                            in1=_bc(g8b, [g8b.ap[0], [8, nr], [0, 5], [1, 8]]),
                            op=mybir.AluOpType.mult)
                        nc.vector.tensor_copy(
                            out=bass.AP(bb.tensor, bb.offset + 40,
                                        [bb.ap[0], [2 * ROW, nr], [1, 1]]),
                            in_=gg[:])
                    # accumulate rounds into per-tile psum
                    for (t, a, n, tr0, done) in segs:
                        if t not in psums:
                            psums[t] = ps_acc.tile([128, ROW1], FP, tag="acc",
                                                   name=f"ps{conv}_{t}")
                        pt = psums[t]
                        for r_ in range(n):
                            nc.tensor.matmul(
                                out=pt[:, :ROW], lhsT=idb[:, :128],
                                rhs=bass.AP(bb.tensor,
                                            bb.offset + (a - r0 + r_) * 2 * ROW,
                                            [bb.ap[0], [1, ROW]]),
                                start=(tr0 + r_ == 0), stop=done and r_ == n - 1)
                        if done:
                            yield t, pt
                            del psums[t]

            agt0 = 0
            for t, pt in conv_pass(1, tab1, 0):
                nd = work.tile([128, ROW1], FP, tag="nd1", name="nd1")
                nc.vector.tensor_copy(out=nd[:], in_=pt[:, :ROW1])
                den = work.tile([128, 8], FP, tag="den1", name="den1")
                nc.vector.tensor_scalar(den[:], nd[:, 64:72], 1e-16, None,
                                        mybir.AluOpType.max)
                rec = work.tile([128, 8], FP, tag="rec1", name="rec1")
                nc.vector.reciprocal(rec[:], den[:])
                h1 = work.tile([128, D1], FP, tag="h1", name="h1")
                rb = rec[:]
                h1v = h1[:]
                ndv = nd[:]
                nc.vector.tensor_tensor(
                    out=_bc(h1v, [h1v.ap[0], [8, 8], [1, 8]]),
                    in0=_bc(ndv, [ndv.ap[0], [8, 8], [1, 8]]),
                    in1=_bc(rb, [rb.ap[0], [0, 8], [1, 8]]),
                    op=mybir.AluOpType.mult)
                nc.vector.tensor_tensor(out=h1[:], in0=h1[:], in1=b1t[:],
                                        op=mybir.AluOpType.add)
                nc.vector.tensor_scalar(h1[:], h1[:], 0.0, None,
                                        mybir.AluOpType.max)
                ptr = ps_e.tile([64, 128], FP, tag="tr", name="ptr")
                nc.tensor.transpose(out=ptr[:], in_=h1[:], identity=idf[:])
                h1T = work.tile([64, 128], BF, tag="h1T", name="h1T")
                nc.vector.tensor_copy(out=h1T[:], in_=ptr[:])
                pf2 = ps_e.tile([128, 42], FP, tag="pf2", name="pf2")
                nc.tensor.matmul(out=pf2[:], lhsT=h1T[:], rhs=w2[:],
                                 start=True, stop=True)
                nc.vector.tensor_copy(out=ad2[:, t:t + 1], in_=pf2[:, 41:42])
                st2 = work.tile([128, 42], BF, tag="st2", name="st2")
                nc.vector.tensor_copy(out=st2[:], in_=pf2[:])
                nc.sync.dma_start(out=shard2[t * 64:(t + 1) * 64, 0:ROW2],
                                  in_=st2[0:64, 0:ROW2])
                nc.sync.dma_start(out=shard2[t * 64:(t + 1) * 64, ROW2:2 * ROW2],
                                  in_=st2[64:128, 0:ROW2])
                # allgather each finished 7-tile piece into its contiguous
                # block: all but the last piece overlaps conv1's gathers
                if t % 7 == 6:
                    i = t // 7
                    nc.gpsimd.collective_compute(
                        "AllGather", mybir.AluOpType.bypass,
                        replica_groups=[list(range(NCORES))],
                        ins=[shard2[i * 448:(i + 1) * 448, :]],
                        outs=[tab2v[i * 3584:(i + 1) * 3584, :]])

            # patch all fake rows' alpha_s2 (pair positions, same all cores)
            nc.sync.dma_start(out=tab2v[21930:21931, 81:82], in_=negt[:1])
            nc.sync.dma_start(out=tab2v[24592:24640, 81:82], in_=negt[:48])
            nc.sync.dma_start(out=tab2v[25025:25088, 40:41], in_=negt[:63])
            nc.sync.dma_start(out=tab2v[25024:25088, 81:82], in_=negt[:64])

            for t, pt in conv_pass(2, tab2v, NW2):
                nd = work.tile([128, ROW2], FP, tag="nd2", name="nd2")
                nc.vector.tensor_copy(out=nd[:], in_=pt[:, :ROW2])
                den = work.tile([128, 1], FP, tag="den2", name="den2")
                nc.vector.tensor_scalar(den[:], nd[:, 40:41], 1e-16, None,
                                        mybir.AluOpType.max)
                rec = work.tile([128, 1], FP, tag="rec2", name="rec2")
                nc.vector.reciprocal(rec[:], den[:])
                o2 = work.tile([128, NC_], FP, tag="o2", name="o2")
                nc.vector.tensor_scalar(o2[:], nd[:, 0:40], rec[:, 0:1], None,
                                        mybir.AluOpType.mult)
                nc.vector.tensor_tensor(out=o2[:], in0=o2[:], in1=b2t[:],
                                        op=mybir.AluOpType.add)
                mx = work.tile([128, 1], FP, tag="mx", name="mx")
                nc.vector.tensor_reduce(mx[:], o2[:], mybir.AxisListType.X,
                                        mybir.AluOpType.max)
                nc.vector.tensor_scalar(o2[:], o2[:], mx[:, 0:1], None,
                                        mybir.AluOpType.subtract)
                ex = work.tile([128, NC_], FP, tag="ex", name="ex")
                sm = work.tile([128, 1], FP, tag="sm", name="sm")
                nc.scalar.activation(ex[:], o2[:], AF.Exp, accum_out=sm[:])
                ls = work.tile([128, 1], FP, tag="ls", name="ls")
                nc.scalar.activation(ls[:], sm[:], AF.Ln)
                nc.vector.tensor_scalar(o2[:], o2[:], ls[:, 0:1], None,
                                        mybir.AluOpType.subtract)
                nc.sync.dma_start(out=out[t], in_=o2[:])

    nc.finalize()
    return nc


# --------------------------------------------------------------------------
# exec: compile once, keep inputs device-resident, chain executes so the
# axon RPC latency (~77ms round-trip regardless of kernel) pipelines away.
# HW time estimate = steady-state throughput: total wall / K chained runs.
# --------------------------------------------------------------------------

def _run_and_time(nc, in_maps, n_reps=800, n_chain=1):
    import time
    import jax
    from jax.sharding import Mesh, PartitionSpec, NamedSharding
    from jax.experimental.shard_map import shard_map
    from concourse import bass2jax

    bass2jax.install_neuronx_cc_hook()
    pname = nc.partition_id_tensor.name if nc.partition_id_tensor else None
    in_names, out_names, out_avals, zero_outs = [], [], [], []
    for alloc in nc.m.functions[0].allocations:
        if not isinstance(alloc, mybir.MemoryLocationSet):
            continue
        name = alloc.memorylocations[0].name
        if alloc.kind == "ExternalInput":
            if name != pname:
                in_names.append(name)
        elif alloc.kind == "ExternalOutput":
            out_names.append(name)
            shape = tuple(alloc.tensor_shape)
            dt = mybir.dt.np(alloc.dtype)
            out_avals.append(jax.core.ShapedArray(shape, dt))
            zero_outs.append(np.zeros(shape, dt))
    n_params = len(in_names)
    all_in = in_names + out_names + ([pname] if pname else [])

    def _body(*args):
        # n_chain back-to-back executions in one XLA program (fori_loop with
        # the bind as the body): each iteration's outputs become the next
        # iteration's scratch output buffers, so a single host dispatch runs
        # the kernel n_chain times device-side with zero host involvement.
        ins = list(args[:n_params])
        outs = tuple(args[n_params:])
        pid = [bass2jax.partition_id_tensor()] if pname is not None else []

        def it(_, carry):
            return tuple(bass2jax._bass_exec_p.bind(
                *ins, *carry, *pid,
                out_avals=tuple(out_avals), in_names=tuple(all_in),
                out_names=tuple(out_names), lowering_input_output_aliases=(),
                sim_require_finite=True, sim_require_nnan=True, nc=nc))

        if n_chain == 1:
            return it(0, outs)
        return jax.lax.fori_loop(0, n_chain, it, outs)

    devices = jax.devices()[:NCORES]
    mesh = Mesh(np.asarray(devices), ("core",))
    nin = n_params + len(out_names)
    f = jax.jit(
        shard_map(_body, mesh=mesh, in_specs=(PartitionSpec("core"),) * nin,
                  out_specs=(PartitionSpec("core"),) * len(out_names),
                  check_rep=False),
        donate_argnums=tuple(range(n_params, nin)), keep_unused=True)
    sh = NamedSharding(mesh, PartitionSpec("core"))
    dev_in = [jax.device_put(
        np.concatenate([m[nm] for m in in_maps], axis=0), sh)
        for nm in in_names]
    zput = lambda: [jax.device_put(
        np.zeros((NCORES * z.shape[0], *z.shape[1:]), z.dtype), sh)
        for z in zero_outs]

    outs = f(*dev_in, *zput())            # compile + warm-up; keep results
    jax.block_until_ready(outs)
    res = [np.asarray(o) for o in outs]
    results = [{nm: res[i].reshape(NCORES, *out_avals[i].shape)[c]
                for i, nm in enumerate(out_names)} for c in range(NCORES)]

    # timed: K chained dispatches of n_chain executions each, donating the
    # previous outputs as scratch
    best = None
    for _ in range(3):
        cur = f(*dev_in, *zput())
        jax.block_until_ready(cur)
        t0 = time.perf_counter()
        for _ in range(n_reps):
            cur = f(*dev_in, *cur)
        jax.block_until_ready(cur)
        dt = (time.perf_counter() - t0) / (n_reps * n_chain)
        best = dt if best is None else min(best, dt)
    return results, int(best * 1e9)


# --------------------------------------------------------------------------
# host entry
# --------------------------------------------------------------------------

def kernel(x, edge_index, W1, as1, ad1, b1, W2, as2, ad2, b2):
    x = np.asarray(x, np.float32)
    ei = np.asarray(edge_index)
    W1 = np.asarray(W1, np.float32); as1 = np.asarray(as1, np.float32)
    ad1 = np.asarray(ad1, np.float32); b1 = np.asarray(b1, np.float32)
    W2 = np.asarray(W2, np.float32); as2 = np.asarray(as2, np.float32)
    ad2 = np.asarray(ad2, np.float32); b2 = np.asarray(b2, np.float32)

    plan = _plan(ei)
    newid, order = plan["newid"], plan["order"]

    # W1ext: [128, 80] = [W1 c-major | W1@as1_h | W1@ad1_h]
    W1cm = W1.reshape(F_IN, H, C1).transpose(0, 2, 1).reshape(F_IN, D1)
    Was = np.stack([W1[:, h * C1:(h + 1) * C1] @ as1[h] for h in range(H)], 1)
    Wad = np.stack([W1[:, h * C1:(h + 1) * C1] @ ad1[h] for h in range(H)], 1)
    W1e = np.concatenate([W1cm, Was, Wad], axis=1).astype(bf16)

    # x_fake: alpha_s1 = ANEG (all heads), alpha_d1 = 0
    M = np.concatenate([Was, Wad], axis=1)                    # [128, 16]
    rhs = np.concatenate([np.full(8, ANEG), np.zeros(8)])
    v = M @ np.linalg.solve(M.T @ M, rhs)
    xT_all = np.zeros((F_IN, NTOT), np.float32)
    xT_all[:, newid] = x.T
    fake = np.setdiff1d(np.arange(NTOT), newid)
    xT_all[:, fake] = v[:, None]
    xT_all = xT_all.astype(bf16)

    # conv2 fake-head col permutation: orig col o = h*5+c ; new j = c*8+h
    sig = np.empty(NC_, np.int64)
    for hh in range(8):
        for cc in range(5):
            sig[cc * 8 + hh] = hh * 5 + cc
    W2p = W2[:, sig]
    W2ex = np.concatenate([W2p, W2 @ as2[0][:, None], W2 @ ad2[0][:, None]],
                          axis=1)                             # [64, 42]
    # h1 columns are c-major (c*8+h); permute W2ext rows to match
    rowperm = np.empty(D1, np.int64)
    for hh in range(H):
        for cc in range(C1):
            rowperm[cc * 8 + hh] = hh * C1 + cc
    W2ex = W2ex[rowperm].astype(bf16)

    b1cm = b1.reshape(H, C1).T.reshape(D1)
    b1r = np.tile(b1cm, (128, 1)).astype(np.float32)
    b2r = np.tile(b2[sig], (128, 1)).astype(np.float32)

    nc = _build(plan)
    in_maps = []
    for c in range(NCORES):
        in_maps.append({
            "xT": xT_all,
            "xTo": np.ascontiguousarray(xT_all[:, c * NSH:(c + 1) * NSH]),
            "W1e": W1e, "W2e": W2ex, "b1r": b1r, "b2r": b2r,
            "idx": np.ascontiguousarray(plan["idx"][c]),
            "mskd": np.ascontiguousarray(plan["msk"][c]),
        })
    results, exec_ns = _run_and_time(nc, in_maps)
    global _LAST_EXEC_NS
    _LAST_EXEC_NS = exec_ns

    out_full = np.zeros((N, NC_), np.float32)
    nid = newid
    core = nid // NSH
    rem = nid % NSH
    tt, ll = rem // 128, rem % 128
    for c in range(NCORES):
        m = core == c
        dev = results[c]["out"]                        # [49, 128, 40]
        out_full[np.where(m)[0]] = dev[tt[m], ll[m]]
    # un-permute columns (device col j holds class sig[j])
    inv = np.empty(NC_, np.int64)
    inv[sig] = np.arange(NC_)
    out_full = out_full[:, inv]
    return out_full


_LAST_EXEC_NS = None

if __name__ == "__main__":
    import pickle
    inputs = pickle.load(open("inputs.pkl", "rb"))
    outp = kernel(**{k: np.asarray(v) for k, v in inputs.items()})
    exp = np.load("expected.npy")
    rel = np.linalg.norm(outp - exp) / np.linalg.norm(exp)
    print("rel:", rel)
